# revision 9
# baseline (speedup 1.0000x reference)
"""AqlmOFTLinear distributed Trainium2 kernel (8 NeuronCores).

Strategy:
  - Data-parallel over tokens for x (2048 tokens/core, host pre-transposed to
    feature-major), tensor-parallel dequant of the AQLM weight (512 out rows
    per core) with a chunked AllGather of the rotated weight V = BD @ W^T.
  - AQLM dequant on-device via gpsimd.dma_gather from a paired codebook table
    [32768, 128]bf16 where entry q = [cb[q] | cb[q+32768] | pad]; the int16
    index is (code & 32767) and bit 15 selects the half via copy_predicated.
  - OFT Cayley transform on-device: Q^T = (I+S^8)(I+S^4)(I+S^2)(I-S) with the
    128 32x32 blocks packed 4-up into block-diagonal 128x128 f32 matmuls.
  - Main matmul in bf16: out^T[o, t] = sum_i V[i, o] * x[t, i], PSUM-accum
    over 32 k-chunks, bias fused into the PSUM->SBUF evacuation.
"""

import os
import sys

import numpy as np

sys.path.insert(0, "/opt/trn_rl_repo")

import ml_dtypes

BF16 = ml_dtypes.bfloat16

N_CORES = 8
IN_F = 4096
OUT_F = 4096
TOK = 16384
TOK_PC = TOK // N_CORES          # 2048 tokens per core
OUT_PC = OUT_F // N_CORES        # 512 out-features per core
GROUP = 8
N_G = IN_F // GROUP              # 512 groups
CB = 65536
HALF_CB = CB // 2                # 32768 table entries
ELEM = 128                       # bf16 elems per table entry (256B)
N_IC = IN_F // 128               # 32 input-feature chunks
GC_G = 16                        # groups per gather call
N_GC = (N_G // 4) // GC_G        # 32 gather calls per o-chunk? no: per oc
# per o-chunk (128 o rows) we cover all 512 groups in calls of GC_G groups
CALLS_PER_OC = N_G // GC_G       # 32 calls, each 128 o x 16 g = 2048 idxs
NIDX = 128 * GC_G                # 2048 indices per gather call

_BUILD_CACHE = {}
LAST_RESULT = None


def _build_nc():
    from concourse import bacc, bass, mybir, tile

    f32 = mybir.dt.float32
    bf16 = mybir.dt.bfloat16
    i16 = mybir.dt.int16

    nc = bacc.Bacc(num_devices=N_CORES)

    # ---- DRAM parameters (per-core shards supplied via in_maps) ----
    xT_d = nc.declare_dram_parameter("xT", [IN_F, TOK_PC], f32, isOutput=False)
    table_d = nc.declare_dram_parameter("table", [HALF_CB, ELEM], bf16, isOutput=False)
    idx_d = nc.declare_dram_parameter("idx", [4 * CALLS_PER_OC, 128, NIDX // 16], i16, isOutput=False)
    u8 = mybir.dt.uint8
    mask_d = nc.declare_dram_parameter("mask", [4 * CALLS_PER_OC, 128, GC_G * GROUP], u8, isOutput=False)
    scales_d = nc.declare_dram_parameter("scales_p", [128, 4], f32, isOutput=False)
    bias_d = nc.declare_dram_parameter("bias_p", [128, 32], f32, isOutput=False)
    rbd_d = nc.declare_dram_parameter("rbd", [N_IC, 128, 128], f32, isOutput=False)
    identf_d = nc.declare_dram_parameter("identf", [128, 128], f32, isOutput=False)
    identb_d = nc.declare_dram_parameter("identb", [128, 128], bf16, isOutput=False)
    outT_d = nc.declare_dram_parameter("outT", [OUT_F, TOK_PC], f32, isOutput=True)

    # ---- internal DRAM for the collective (4 o-chunks of 128) ----
    cc_in = [nc.dram_tensor(f"cc_in{j}", [IN_F, 128], bf16) for j in range(4)]
    cc_out = [
        nc.dram_tensor(f"cc_out{j}", [N_CORES * IN_F, 128], bf16, addr_space="Shared")
        for j in range(4)
    ]
    rg = [list(range(N_CORES))]

    with tile.TileContext(nc) as tc:
        with (
            tc.tile_pool(name="const", bufs=1) as constp,
            tc.tile_pool(name="qt", bufs=1) as qtp,
            tc.tile_pool(name="xh", bufs=1) as xhp,
            tc.tile_pool(name="vs", bufs=2) as vsp,
            tc.tile_pool(name="ob", bufs=2) as obp,
            tc.tile_pool(name="cay", bufs=6) as cayp,
            tc.tile_pool(name="deq", bufs=1) as deqp,
            tc.tile_pool(name="deq2", bufs=2) as deq2p,
            tc.tile_pool(name="psA", bufs=2, space="PSUM") as psA,
            tc.tile_pool(name="psV", bufs=2, space="PSUM") as psVp,
            tc.tile_pool(name="psB", bufs=2, space="PSUM") as psB,
        ):
            # ---- constants ----
            identf = constp.tile([128, 128], f32)
            nc.sync.dma_start(out=identf[:], in_=identf_d[:])
            identb = constp.tile([128, 128], bf16)
            nc.sync.dma_start(out=identb[:], in_=identb_d[:])
            scales_sb = constp.tile([128, 4], f32)
            nc.sync.dma_start(out=scales_sb[:], in_=scales_d[:])
            bias_sb = constp.tile([128, 32], f32)
            nc.sync.dma_start(out=bias_sb[:], in_=bias_d[:])
            ident4 = constp.tile([128, 4, 128], f32)
            for k in range(4):
                nc.vector.tensor_copy(ident4[:, k, :], identf[:])

            qt_sb = qtp.tile([128, N_IC, 128], bf16)  # Q^T block-diag chunks
            nidx_reg = nc.gpsimd.to_reg(NIDX)  # shared across all gather calls

            # ================= Phase A0: Cayley =================
            # per group of 4 chunks (batched DVE, per-chunk matmuls)
            for g in range(8):
                rbd_sb = cayp.tile([128, 4, 128], f32, tag="cay")
                nc.sync.dma_start(
                    out=rbd_sb[:],
                    in_=rbd_d[g * 4:(g + 1) * 4, :, :].rearrange("c p f -> p c f"),
                )
                psT = psA.tile([128, 4, 128], f32, tag="ps")
                for k in range(4):
                    nc.tensor.transpose(psT[:, k, :], rbd_sb[:, k, :], identf[:])
                tmp = cayp.tile([128, 4, 128], f32, tag="cay")
                nc.vector.tensor_scalar_mul(tmp[:], rbd_sb[:], 0.5)
                S = cayp.tile([128, 4, 128], f32, tag="cay")
                nc.vector.scalar_tensor_tensor(
                    S[:], psT[:], -0.5, tmp[:],
                    mybir.AluOpType.mult, mybir.AluOpType.add,
                )
                negS = cayp.tile([128, 4, 128], f32, tag="cay")
                nc.vector.tensor_scalar_mul(negS[:], S[:], -1.0)
                P1T = cayp.tile([128, 4, 128], f32, tag="cay")  # I - S
                nc.vector.scalar_tensor_tensor(
                    P1T[:], S[:], -1.0, ident4[:],
                    mybir.AluOpType.mult, mybir.AluOpType.add,
                )
                ps2 = psA.tile([128, 4, 128], f32, tag="ps")
                for k in range(4):
                    nc.tensor.matmul(ps2[:, k, :], negS[:, k, :], S[:, k, :])
                S2 = cayp.tile([128, 4, 128], f32, tag="cay")
                nc.vector.tensor_copy(S2[:], ps2[:])
                P2 = cayp.tile([128, 4, 128], f32, tag="cay")  # I + S^2
                nc.vector.tensor_tensor(P2[:], S2[:], ident4[:], mybir.AluOpType.add)
                ps4 = psA.tile([128, 4, 128], f32, tag="ps")
                for k in range(4):
                    nc.tensor.matmul(ps4[:, k, :], S2[:, k, :], S2[:, k, :])
                S4 = cayp.tile([128, 4, 128], f32, tag="cay")
                nc.vector.tensor_copy(S4[:], ps4[:])
                P3 = cayp.tile([128, 4, 128], f32, tag="cay")  # I + S^4
                nc.vector.tensor_tensor(P3[:], S4[:], ident4[:], mybir.AluOpType.add)
                ps8 = psA.tile([128, 4, 128], f32, tag="ps")
                for k in range(4):
                    nc.tensor.matmul(ps8[:, k, :], S4[:, k, :], S4[:, k, :])
                P4 = cayp.tile([128, 4, 128], f32, tag="cay")  # I + S^8
                nc.vector.scalar_tensor_tensor(
                    P4[:], ps8[:], 1.0, ident4[:],
                    mybir.AluOpType.mult, mybir.AluOpType.add,
                )
                # T1 = (I - S)^2: Q^T needs TWO (I-S) factors —
                # (I+S)(I+S^2)(I+S^4)(I+S^8) alone is only (I-S)^{-1}'s series.
                P1 = cayp.tile([128, 4, 128], f32, tag="cay")  # I + S
                nc.vector.tensor_tensor(P1[:], S[:], ident4[:], mybir.AluOpType.add)
                psT1 = psA.tile([128, 4, 128], f32, tag="ps")
                for k in range(4):
                    nc.tensor.matmul(psT1[:, k, :], P1[:, k, :], P1T[:, k, :])
                T1 = cayp.tile([128, 4, 128], f32, tag="cay")
                nc.vector.tensor_copy(T1[:], psT1[:])
                psb1 = psA.tile([128, 4, 128], f32, tag="ps")
                for k in range(4):
                    nc.tensor.matmul(psb1[:, k, :], P2[:, k, :], T1[:, k, :])
                B1 = cayp.tile([128, 4, 128], f32, tag="cay")
                nc.vector.tensor_copy(B1[:], psb1[:])
                psb2 = psA.tile([128, 4, 128], f32, tag="ps")
                for k in range(4):
                    nc.tensor.matmul(psb2[:, k, :], P3[:, k, :], B1[:, k, :])
                B2 = cayp.tile([128, 4, 128], f32, tag="cay")
                nc.vector.tensor_copy(B2[:], psb2[:])
                psb3 = psA.tile([128, 4, 128], f32, tag="ps")
                for k in range(4):
                    nc.tensor.matmul(psb3[:, k, :], P4[:, k, :], B2[:, k, :])
                # Q^T chunk, cast to bf16
                nc.vector.tensor_copy(qt_sb[:, g * 4:(g + 1) * 4, :], psb3[:])

            # ============== Phase A1: dequant + rotate + AllGather ==============
            for oc in range(4):
                wc = deqp.tile([128, IN_F], bf16, tag="wc")  # W rows [128 o, 4096 i]
                for gc in range(CALLS_PER_OC):
                    call = oc * CALLS_PER_OC + gc
                    idx_sb = deq2p.tile([128, NIDX // 16], i16, tag="idx")
                    nc.sync.dma_start(out=idx_sb[:], in_=idx_d[call, :, :])
                    G = deq2p.tile([128, GC_G, ELEM], bf16, tag="G")
                    nc.gpsimd.dma_gather(
                        G[:], table_d[:, :], idx_sb[:],
                        num_idxs=NIDX, num_idxs_reg=nidx_reg, elem_size=ELEM,
                        single_packet=False,
                    )
                    msk = deq2p.tile([128, GC_G, GROUP], u8, tag="msk")
                    nc.sync.dma_start(
                        out=msk[:],
                        in_=mask_d[call, :, :].rearrange("p (g j) -> p g j", j=GROUP),
                    )
                    wv = wc[:, gc * 128:(gc + 1) * 128].rearrange(
                        "p (g j) -> p g j", j=GROUP
                    )
                    sc = scales_sb[:, oc:oc + 1]
                    nc.vector.tensor_scalar_mul(wv, G[:, :, 0:GROUP], sc)
                    tmp8 = deq2p.tile([128, GC_G, GROUP], bf16, tag="tmp8")
                    nc.vector.tensor_scalar_mul(tmp8[:], G[:, :, GROUP:2 * GROUP], sc)
                    nc.vector.copy_predicated(wv, msk[:], tmp8[:])
                # transpose wc -> wt_oc [i'' part, o free]
                wt_oc = deqp.tile([128, N_IC, 128], bf16, tag="wt")
                for icg in range(8):
                    psT2 = psA.tile([128, 4, 128], bf16, tag="ps")
                    for k in range(4):
                        ic = icg * 4 + k
                        nc.tensor.transpose(
                            psT2[:, k, :], wc[:, ic * 128:(ic + 1) * 128], identb[:]
                        )
                    nc.vector.tensor_copy(wt_oc[:, icg * 4:(icg + 1) * 4, :], psT2[:])
                # rotate: V[i, o] = sum_i'' BD[i, i''] * W^T[i'', o]
                for ic in range(N_IC):
                    psVt = psVp.tile([128, 128], f32, tag="psv")
                    nc.tensor.matmul(psVt[:], qt_sb[:, ic, :], wt_oc[:, ic, :])
                    vout = deq2p.tile([128, 128], bf16, tag="vout")
                    nc.vector.tensor_copy(vout[:], psVt[:])
                    nc.sync.dma_start(
                        out=cc_in[oc][ic * 128:(ic + 1) * 128, :], in_=vout[:]
                    )
                nc.gpsimd.collective_compute(
                    "AllGather",
                    mybir.AluOpType.bypass,
                    replica_groups=rg,
                    ins=[cc_in[oc][:, :].opt()],
                    outs=[cc_out[oc][:, :].opt()],
                )

            # ================= Phase B: main matmul =================
            xh = xhp.tile([128, N_IC, TOK_PC], bf16)
            for ic in range(N_IC):
                # SWDGE cast f32 -> bf16 during DMA
                nc.gpsimd.dma_start(
                    out=xh[:, ic, :], in_=xT_d[ic * 128:(ic + 1) * 128, :]
                )
            for s4 in range(4):
                for r in range(N_CORES):
                    vsb = vsp.tile([128, N_IC, 128], bf16, tag="vsb")
                    nc.sync.dma_start(
                        out=vsb[:],
                        in_=cc_out[s4][r * IN_F:(r + 1) * IN_F, :].rearrange(
                            "(ic p) o -> p ic o", p=128
                        ),
                    )
                    for tb in range(TOK_PC // 1024):
                        ps = psB.tile([128, 1024], f32, tag="psb")
                        for ic in range(N_IC):
                            nc.tensor.matmul(
                                ps[:, 0:512],
                                vsb[:, ic, :],
                                xh[:, ic, tb * 1024:tb * 1024 + 512],
                                start=(ic == 0),
                                stop=(ic == N_IC - 1),
                            )
                            nc.tensor.matmul(
                                ps[:, 512:1024],
                                vsb[:, ic, :],
                                xh[:, ic, tb * 1024 + 512:(tb + 1) * 1024],
                                start=(ic == 0),
                                stop=(ic == N_IC - 1),
                            )
                        ob = obp.tile([128, 1024], f32, tag="ob")
                        s_glob = r * 4 + s4
                        nc.vector.tensor_scalar_add(
                            ob[:], ps[:], bias_sb[:, s_glob:s_glob + 1]
                        )
                        nc.sync.dma_start(
                            out=outT_d[
                                s_glob * 128:(s_glob + 1) * 128,
                                tb * 1024:(tb + 1) * 1024,
                            ],
                            in_=ob[:],
                        )
    nc.compile()
    return nc


def _host_prep(x, oft_r, codes, codebooks, scales, bias):
    """Shard + repack all inputs for the 8 cores."""
    xt = np.ascontiguousarray(np.asarray(x, dtype=np.float32).reshape(TOK, IN_F))
    codes2 = np.asarray(codes, dtype=np.int64)[:, :, 0]        # [4096, 512]
    cb = np.asarray(codebooks, dtype=np.float32)[0]            # [65536, 8]
    scales = np.asarray(scales, dtype=np.float32).reshape(OUT_F)
    bias = np.asarray(bias, dtype=np.float32).reshape(OUT_F)
    R = np.asarray(oft_r, dtype=np.float32)                    # [128, 32, 32]

    table = np.zeros((HALF_CB, ELEM), dtype=BF16)
    table[:, 0:GROUP] = cb[:HALF_CB].astype(BF16)
    table[:, GROUP:2 * GROUP] = cb[HALF_CB:].astype(BF16)

    rbd = np.zeros((N_IC, 128, 128), dtype=np.float32)
    Rb = R.reshape(N_IC, 4, 32, 32)
    for a in range(4):
        rbd[:, a * 32:(a + 1) * 32, a * 32:(a + 1) * 32] = Rb[:, a]

    identf = np.eye(128, dtype=np.float32)
    identb = np.eye(128, dtype=BF16)
    bias_p = np.ascontiguousarray(bias.reshape(32, 128).T)     # [128, 32]

    in_maps = []
    for r in range(N_CORES):
        xT = np.ascontiguousarray(xt[r * TOK_PC:(r + 1) * TOK_PC].T)  # [4096, 2048]
        c = codes2[r * OUT_PC:(r + 1) * OUT_PC]                # [512 o, 512 g]
        idx14 = (c & 32767).astype(np.int16)
        hi = (c >> 15).astype(np.float32)                      # 0/1 mask
        # gather call (oc, gc): idx stream n = gl*128 + ol
        idx_c = idx14.reshape(4, 128, CALLS_PER_OC, GC_G)      # [oc, ol, gc, gl]
        stream = np.ascontiguousarray(idx_c.transpose(0, 2, 3, 1)).reshape(
            4 * CALLS_PER_OC, NIDX
        )
        wrapped = stream.reshape(4 * CALLS_PER_OC, NIDX // 16, 16).transpose(0, 2, 1)
        idx_dram = np.ascontiguousarray(
            np.broadcast_to(
                wrapped[:, None, :, :], (4 * CALLS_PER_OC, 8, 16, NIDX // 16)
            ).reshape(4 * CALLS_PER_OC, 128, NIDX // 16)
        )
        # mask[call, p, gl*8 + j] = hi[oc*128+p, gc*16+gl]
        m_c = hi.reshape(4, 128, CALLS_PER_OC, GC_G)           # [oc, p, gc, gl]
        m_t = m_c.transpose(0, 2, 1, 3)                        # [oc, gc, p, gl]
        mask_dram = np.ascontiguousarray(
            np.repeat(m_t.reshape(4 * CALLS_PER_OC, 128, GC_G, 1), GROUP, axis=3)
        ).reshape(4 * CALLS_PER_OC, 128, GC_G * GROUP).astype(np.uint8)
        sc = scales[r * OUT_PC:(r + 1) * OUT_PC]
        scales_p = np.ascontiguousarray(sc.reshape(4, 128).T)  # [128, 4]
        in_maps.append(
            dict(
                xT=xT,
                table=table,
                idx=idx_dram,
                mask=mask_dram,
                scales_p=scales_p,
                bias_p=bias_p,
                rbd=rbd,
                identf=identf,
                identb=identb,
            )
        )
    return in_maps


def kernel(x, oft_r, codes, codebooks, scales, bias):
    global LAST_RESULT
    from concourse.bass_utils import run_bass_kernel_spmd

    if "nc" not in _BUILD_CACHE:
        _BUILD_CACHE["nc"] = _build_nc()
    nc = _BUILD_CACHE["nc"]

    in_maps = _host_prep(x, oft_r, codes, codebooks, scales, bias)
    trace = bool(int(os.environ.get("AQLM_TRACE", "0")))
    res = run_bass_kernel_spmd(nc, in_maps, core_ids=list(range(N_CORES)), trace=trace)
    LAST_RESULT = res

    out = np.empty((TOK, OUT_F), dtype=np.float32)
    for r in range(N_CORES):
        out[r * TOK_PC:(r + 1) * TOK_PC, :] = res.results[r]["outT"].T
    return out.reshape(4, 4096, 4096).astype(np.asarray(x).dtype)
